# revision 7
# baseline (speedup 1.0000x reference)
"""Trainium2 Bass kernel for the LSTM autoencoder problem.

Sharding: data-parallel over batch (B=512 -> 64 per core, 8 cores),
weights replicated. Everything on-device runs in "feature-major" layout:
features on SBUF partitions, batch on the free dim, so the recurrent
matmuls are lhsT=weight-tile [128,128] x rhs=state [128,64] -> PSUM.

Key algebraic facts used:
  * encoder layer1 sees x==h, so z1 = h @ (W1+U1)        (one matmul)
  * relu(c) == c since c >= 0 inductively (g=relu>=0, i,f=sigmoid>0)
  * decoder feeds out_t back in, so for t>=1:
      z_{t+1} = h_t @ (dec_U + out_W @ dec_W) + (dec_b + out_b @ dec_W)
    which removes the dense layer from the critical path.
"""

import os
import sys

import numpy as np

for _p in ("/opt/trn_rl_repo", "/root/.axon_site/_ro/trn_rl_repo"):
    if os.path.isdir(_p) and _p not in sys.path:
        sys.path.insert(0, _p)

import ml_dtypes

B, T, D, L = 512, 512, 128, 256
NCORES = 8
BL = B // NCORES  # 64 batch rows per core
NM = 8            # m-chunks of 4L=1024 (128 each)
BF16 = ml_dtypes.bfloat16

# Test hook: reduced number of timesteps (full problem uses 512).
T_RUN = int(os.environ.get("LSTM_T_RUN", str(T)))
UNROLL = 16

_CACHE = {}


def _build_nc(t_run):
    import concourse.bass as bass
    import concourse.bacc as bacc
    import concourse.mybir as mybir
    import concourse.tile as tile

    fp32 = mybir.dt.float32
    bf16 = mybir.dt.bfloat16
    SIG = mybir.ActivationFunctionType.Sigmoid
    MULT = mybir.AluOpType.mult
    MAX = mybir.AluOpType.max

    nc = bacc.Bacc("TRN2", target_bir_lowering=False)

    # ---- external I/O (per core) ----
    xt = nc.declare_dram_parameter("xt", [128, t_run * BL], bf16, isOutput=False)
    dec0 = nc.declare_dram_parameter("dec0", [128, BL], bf16, isOutput=False)
    w0t = nc.declare_dram_parameter("w0t", [128, 1 * NM * 128], bf16, isOutput=False)
    u0t = nc.declare_dram_parameter("u0t", [128, 2 * NM * 128], bf16, isOutput=False)
    w1u1t = nc.declare_dram_parameter("w1u1t", [128, 2 * NM * 128], bf16, isOutput=False)
    decwt = nc.declare_dram_parameter("decwt", [128, 1 * NM * 128], bf16, isOutput=False)
    decut = nc.declare_dram_parameter("decut", [128, 2 * NM * 128], bf16, isOutput=False)
    wcombt = nc.declare_dram_parameter("wcombt", [128, 2 * NM * 128], bf16, isOutput=False)
    outwt = nc.declare_dram_parameter("outwt", [128, 2 * 128], bf16, isOutput=False)
    outd = nc.declare_dram_parameter("outT", [128, t_run * BL], fp32, isOutput=True)

    with tile.TileContext(nc) as tc:
        with (
            tc.tile_pool(name="singles", bufs=1) as singles,
            tc.tile_pool(name="xin", bufs=3) as xin,
            tc.tile_pool(name="gates", bufs=3) as gates,
            tc.tile_pool(name="tmps", bufs=3) as tmps,
            tc.tile_pool(name="outs", bufs=2) as outs,
            tc.tile_pool(name="zps", bufs=2, space="PSUM") as zps,
            tc.tile_pool(name="ops", bufs=2, space="PSUM") as ops,
        ):
            # ---- load weights into SBUF ----
            def wload(param, ntiles):
                t_ = singles.tile([128, ntiles * 128], bf16, tag=param.name)
                nc.sync.dma_start(out=t_[:], in_=param[:])
                return t_

            sb_w0 = wload(w0t, NM)
            sb_u0 = wload(u0t, 2 * NM)
            sb_w1u1 = wload(w1u1t, 2 * NM)
            sb_decw = wload(decwt, NM)
            sb_decu = wload(decut, 2 * NM)
            sb_wcomb = wload(wcombt, 2 * NM)
            sb_outw = wload(outwt, 2)
            sb_dec0 = singles.tile([128, BL], bf16, tag="dec0")
            nc.sync.dma_start(out=sb_dec0[:], in_=dec0[:])

            # ---- recurrent state ----
            h = singles.tile([128, 2 * BL], bf16, tag="h")      # carry h (bf16)
            hmid = singles.tile([128, 2 * BL], bf16, tag="hmid")  # encoder layer0 out
            c = singles.tile([128, 2 * BL], fp32, tag="c")      # cell state fp32
            nc.vector.memset(h[:], 0.0)
            nc.vector.memset(c[:], 0.0)

            # MM emission order: f first (earliest DVE start), o late, g late.
            M_ORDER = [2, 3, 0, 1, 4, 5, 6, 7]

            def lstm_cell(rhs_chunks, lhs_tiles, h_out):
                """One LSTM cell step. rhs_chunks: list of [128, BL] bf16 APs
                (contraction chunks). lhs_tiles: list (same length) of
                (sbuf_weights, tile_base) so lhsT for (kc, m) is
                sbuf[:, (tile_base_kc + m)*128 : ...]. Updates c in place,
                writes h_out (bf16 [128, 2*BL])."""
                nk = len(rhs_chunks)
                z = zps.tile([128, NM * BL], fp32, tag="z")
                for m in M_ORDER:
                    for kc in range(nk):
                        wsb, base = lhs_tiles[kc]
                        lhsT = wsb[:, (base + m) * 128:(base + m + 1) * 128]
                        nc.tensor.matmul(
                            z[:, m * BL:(m + 1) * BL],
                            lhsT,
                            rhs_chunks[kc],
                            start=(kc == 0),
                            stop=(kc == nk - 1),
                        )
                sb_if = gates.tile([128, 4 * BL], bf16, tag="sb_if")
                sb_o = gates.tile([128, 2 * BL], bf16, tag="sb_o")
                # i,f are m-chunks 0..3; o is 6,7; g is 4,5 (kept raw in PSUM)
                nc.scalar.activation(sb_if[:], z[:, 0:4 * BL], SIG)
                nc.scalar.activation(sb_o[:], z[:, 6 * BL:8 * BL], SIG)
                tg = tmps.tile([128, 2 * BL], fp32, tag="tg")
                t2 = tmps.tile([128, 2 * BL], fp32, tag="t2")
                # tg = relu(zg) * i   (i>0 so max-then-mult == i*relu(g))
                nc.vector.scalar_tensor_tensor(
                    tg[:], z[:, 4 * BL:6 * BL], 0.0, sb_if[:, 0:2 * BL], MAX, MULT
                )
                # t2 = f * c ; c = t2 + tg ; h = o * c
                nc.vector.tensor_tensor(t2[:], sb_if[:, 2 * BL:4 * BL], c[:], MULT)
                nc.vector.tensor_tensor(c[:], t2[:], tg[:], mybir.AluOpType.add)
                nc.vector.tensor_tensor(h_out[:], sb_o[:], c[:], MULT)

            def enc_step(xt_rhs):
                lstm_cell(
                    [xt_rhs, h[:, 0:BL], h[:, BL:2 * BL]],
                    [(sb_w0, 0), (sb_u0, 0), (sb_u0, NM)],
                    hmid,
                )
                lstm_cell(
                    [hmid[:, 0:BL], hmid[:, BL:2 * BL]],
                    [(sb_w1u1, 0), (sb_w1u1, NM)],
                    h,
                )

            def dec_step(out_ap, extra_x=None):
                if extra_x is not None:  # first decoder step: x-input + dec_U
                    lstm_cell(
                        [extra_x, h[:, 0:BL], h[:, BL:2 * BL]],
                        [(sb_decw, 0), (sb_decu, 0), (sb_decu, NM)],
                        h,
                    )
                else:  # folded recurrence
                    lstm_cell(
                        [h[:, 0:BL], h[:, BL:2 * BL]],
                        [(sb_wcomb, 0), (sb_wcomb, NM)],
                        h,
                    )
                # out projection: outT = out_W.T @ h  -> [128(D), BL]
                op = ops.tile([128, BL], fp32, tag="op")
                nc.tensor.matmul(op[:], sb_outw[:, 0:128], h[:, 0:BL],
                                 start=True, stop=False)
                nc.tensor.matmul(op[:], sb_outw[:, 128:256], h[:, BL:2 * BL],
                                 start=False, stop=True)
                nc.vector.tensor_copy(out_ap, op[:])

            # ============ encoder (fully unrolled, static APs) ============
            for t_ in range(t_run):
                if t_ % UNROLL == 0:
                    nsub = min(UNROLL, t_run - t_)
                    xg = xin.tile([128, nsub * BL], bf16, tag="xg")
                    nc.sync.dma_start(
                        out=xg[:], in_=xt[:, t_ * BL:(t_ + nsub) * BL]
                    )
                j = t_ % UNROLL
                enc_step(xg[:, j * BL:(j + 1) * BL])

            # ============ decoder (fully unrolled, static APs) ============
            for t_ in range(t_run):
                if t_ % UNROLL == 0:
                    nsub = min(UNROLL, t_run - t_)
                    stage = outs.tile([128, nsub * BL], fp32, tag="stage")
                j = t_ % UNROLL
                dec_step(stage[:, j * BL:(j + 1) * BL],
                         extra_x=sb_dec0[:] if t_ == 0 else None)
                if j == nsub - 1:
                    nc.sync.dma_start(
                        out=outd[:, (t_ - j) * BL:(t_ + 1) * BL], in_=stage[:]
                    )

    nc.compile()
    return nc


def _host_prep(inputs, t_run):
    """Build per-core input maps (numpy only)."""
    f32 = np.float32

    def tile_w(w):  # [K, 4L] -> [128, nk*nm*128] (lhsT tiles along free dim)
        k = w.shape[0]
        nk = k // 128
        nm = w.shape[1] // 128
        return np.ascontiguousarray(
            w.reshape(nk, 128, nm, 128).transpose(1, 0, 2, 3).reshape(128, nk * nm * 128)
        ).astype(BF16)

    w0 = np.asarray(inputs["enc_W0"], f32)
    u0 = np.asarray(inputs["enc_U0"], f32)
    w1u1 = np.asarray(inputs["enc_W1"], f32) + np.asarray(inputs["enc_U1"], f32)
    decw = np.asarray(inputs["dec_W"], f32)
    decu = np.asarray(inputs["dec_U"], f32)
    outw = np.asarray(inputs["out_W"], f32)
    wcomb = decu + outw @ decw

    for bname in ("enc_b0", "enc_b1", "dec_b", "out_b"):
        assert not np.any(np.asarray(inputs[bname])), f"nonzero bias {bname} unsupported"

    wmap = {
        "w0t": tile_w(w0), "u0t": tile_w(u0), "w1u1t": tile_w(w1u1),
        "decwt": tile_w(decw), "decut": tile_w(decu), "wcombt": tile_w(wcomb),
        "outwt": tile_w(outw),
    }

    enc = np.asarray(inputs["encoder_inputs"], f32)
    dec0 = np.asarray(inputs["decoder_inputs"], f32)[:, 0, :]
    in_maps = []
    for cid in range(NCORES):
        bs = slice(cid * BL, (cid + 1) * BL)
        # [BL, T, D] -> [D, T, BL] -> [128, T*BL]
        xt_c = np.ascontiguousarray(
            enc[bs, :t_run, :].transpose(2, 1, 0).reshape(128, t_run * BL)
        ).astype(BF16)
        d0_c = np.ascontiguousarray(dec0[bs, :].T).astype(BF16)
        in_maps.append({"xt": xt_c, "dec0": d0_c, **wmap})
    return in_maps


def _run(inputs, t_run, trace=False):
    from concourse.bass_utils import run_bass_kernel_spmd

    key = t_run
    if key not in _CACHE:
        _CACHE[key] = _build_nc(t_run)
    nc = _CACHE[key]
    in_maps = _host_prep(inputs, t_run)
    res = run_bass_kernel_spmd(nc, in_maps, list(range(NCORES)), trace=trace)
    outs = []
    for cid in range(NCORES):
        o = np.asarray(res.results[cid]["outT"]).reshape(128, t_run, BL)
        outs.append(o.transpose(2, 1, 0))  # -> [BL, t_run, D]
    full = np.concatenate(outs, axis=0).astype(np.float32)
    return full, res


def kernel(**inputs):
    out, _ = _run(inputs, T_RUN, trace=False)
    return out
